# revision 2
# baseline (speedup 1.0000x reference)
"""NaryTreeLSTM Trainium2 kernel.

Strategy: pure data-parallel over batch (B=32768 -> 4096 rows/core on 8
cores). All on-device compute happens in transposed [h, batch] layout so
matmuls contract over the SBUF partition dim; activations are pre-cast to
fp16 host-side (halves DMA, 4x faster PE than fp32). Gate preactivations
accumulate in PSUM (x-path + hsum-path + bias via ACT), nonlinearities run
on the scalar engine straight out of PSUM, and the k-reductions (hsum,
c = i*u + sum_k f_k*cc_k) are single strided DVE reduce instructions.
"""

import sys

sys.path.insert(0, "/opt/trn_rl_repo")

import numpy as np

B, K, I, H = 32768, 4, 256, 256
NCORES = 8
BLOC = B // NCORES  # 4096 batch rows per core
C = 512  # chunk columns (one PSUM bank of fp32)

_cache = {}


def _build(nchunk):
    """Build the per-core Bass program (identical on all cores)."""
    import concourse.bass as bass  # noqa: F401
    import concourse.tile as tile
    from concourse import bacc, mybir

    f16, f32 = mybir.dt.float16, mybir.dt.float32
    AF = mybir.ActivationFunctionType
    X = mybir.AxisListType.X
    ADD = mybir.AluOpType.add

    nc = bacc.Bacc("TRN2", target_bir_lowering=False, debug=False, num_devices=NCORES)

    # DRAM I/O. ax packs, per (chunk, itile): [cx_k0|cx_k1|cx_k2|cx_k3|x]
    # blocks of C columns each, rows = 128 contraction indices.
    ax = nc.dram_tensor("ax", [nchunk, 2, 128, 5 * C], f16, kind="ExternalInput").ap()
    # wcat cols: 0:768 WxiouT (g*256+h), 768:1024 WfT, 1024:2816 UhT
    # ([Ui|Uo|Uu|WfK0..3] at 1024+blk*256+h); rows = contraction index.
    wcat = nc.dram_tensor("wcat", [2, 128, 2816], f16, kind="ExternalInput").ap()
    # bmat cols: 2g+t for g in {i,o,u,f}, t = h-tile
    bmat = nc.dram_tensor("bmat", [128, 8], f32, kind="ExternalInput").ap()
    h_out = nc.dram_tensor("h_out", [256, nchunk * C], f32, kind="ExternalOutput").ap()
    c_out = nc.dram_tensor("c_out", [256, nchunk * C], f32, kind="ExternalOutput").ap()

    with tile.TileContext(nc) as tc:
        import contextlib

        with contextlib.ExitStack() as ctx:
            wpool = ctx.enter_context(tc.tile_pool(name="w", bufs=1))
            apool = ctx.enter_context(tc.tile_pool(name="a", bufs=3))
            gpool = ctx.enter_context(tc.tile_pool(name="g", bufs=1))
            opool = ctx.enter_context(tc.tile_pool(name="o", bufs=2))
            ppool = ctx.enter_context(tc.tile_pool(name="ps", bufs=1, space="PSUM"))

            w_sb = []
            for it in range(2):
                w = wpool.tile([128, 2816], f16, tag=f"w{it}", name=f"w{it}")
                nc.sync.dma_start(w[:], wcat[it])
                w_sb.append(w)
            b_sb = wpool.tile([128, 8], f32, tag="bias", name="b_sb")
            nc.sync.dma_start(b_sb[:], bmat[:])

            def bias(g, ht):
                # g: 0=i, 1=o, 2=u, 3=f
                return b_sb[:, 2 * g + ht : 2 * g + ht + 1]

            def emit_child(c):
                """Child (leaf) phase for chunk c. Returns live tiles."""
                a_sb = []
                for it in range(2):
                    a = apool.tile([128, 5 * C], f16, tag=f"a{it}", name=f"a{it}")
                    nc.sync.dma_start(a[:], ax[c, it])
                    a_sb.append(a)
                ci, co, cu = {}, {}, {}
                for ht in range(2):
                    ci[ht] = gpool.tile([128, 4 * C], f16, tag=f"ci{ht}", bufs=2, name=f"ci{ht}")
                    co[ht] = gpool.tile([128, 4 * C], f16, tag=f"co{ht}", bufs=2, name=f"co{ht}")
                    cu[ht] = gpool.tile([128, 4 * C], f16, tag=f"cu{ht}", bufs=2, name=f"cu{ht}")
                for k in range(4):
                    for ht in range(2):
                        ps = ppool.tile([128, 3 * C], f32, tag="p3", bufs=2, name="ps3")
                        for g in range(3):
                            col = g * 256 + ht * 128
                            for it in range(2):
                                nc.tensor.matmul(
                                    ps[:, g * C : (g + 1) * C],
                                    lhsT=w_sb[it][:, col : col + 128],
                                    rhs=a_sb[it][:, k * C : (k + 1) * C],
                                    start=(it == 0),
                                    stop=(it == 1),
                                )
                        ksl = slice(k * C, (k + 1) * C)
                        nc.scalar.activation(
                            ci[ht][:, ksl], ps[:, 0:C], AF.Sigmoid, bias=bias(0, ht)
                        )
                        nc.scalar.activation(
                            co[ht][:, ksl], ps[:, C : 2 * C], AF.Sigmoid, bias=bias(1, ht)
                        )
                        nc.scalar.activation(
                            cu[ht][:, ksl], ps[:, 2 * C : 3 * C], AF.Tanh, bias=bias(2, ht)
                        )
                ucc, hs = {}, {}
                for ht in range(2):
                    # ucc = [u_node | cc0..cc3]; cc written now, u in node phase
                    ucc[ht] = gpool.tile([128, 5 * C], f16, tag=f"ucc{ht}", bufs=2, name=f"ucc{ht}")
                    nc.vector.tensor_mul(ucc[ht][:, C : 5 * C], ci[ht][:], cu[ht][:])
                    tcc = gpool.tile([128, 4 * C], f16, tag=f"tcc{ht}")
                    nc.scalar.activation(tcc[:], ucc[ht][:, C : 5 * C], AF.Tanh)
                    prod = gpool.tile([128, 4 * C], f16, tag=f"prod{ht}")
                    nc.vector.tensor_mul(prod[:], co[ht][:], tcc[:])
                    hs[ht] = gpool.tile([128, C], f16, tag=f"hs{ht}", bufs=2, name=f"hs{ht}")
                    with nc.allow_low_precision("hsum kept in fp16 for matmul rhs"):
                        nc.vector.tensor_reduce(
                            hs[ht][:],
                            prod[:].rearrange("p (k c) -> p c k", k=4),
                            axis=X,
                            op=ADD,
                        )
                return dict(c=c, a=a_sb, ucc=ucc, hs=hs)

            def emit_node(st):
                c, a_sb, ucc, hs = st["c"], st["a"], st["ucc"], st["hs"]
                xsl = slice(4 * C, 5 * C)
                for ht in range(2):
                    ps = ppool.tile([128, 3 * C], f32, tag="p3", bufs=2, name="ps3")
                    for g in range(3):
                        sl = ps[:, g * C : (g + 1) * C]
                        col = g * 256 + ht * 128
                        for it in range(2):
                            nc.tensor.matmul(
                                sl,
                                lhsT=w_sb[it][:, col : col + 128],
                                rhs=a_sb[it][:, xsl],
                                start=(it == 0),
                                stop=False,
                            )
                        ucol = 1024 + g * 256 + ht * 128
                        for ct in range(2):
                            nc.tensor.matmul(
                                sl,
                                lhsT=w_sb[ct][:, ucol : ucol + 128],
                                rhs=hs[ct][:],
                                start=False,
                                stop=(ct == 1),
                            )
                    ifff = gpool.tile([128, 5 * C], f16, tag=f"ifff{ht}")
                    o_sb = gpool.tile([128, C], f16, tag=f"o{ht}")
                    nc.scalar.activation(
                        ifff[:, 0:C], ps[:, 0:C], AF.Sigmoid, bias=bias(0, ht)
                    )
                    nc.scalar.activation(
                        o_sb[:], ps[:, C : 2 * C], AF.Sigmoid, bias=bias(1, ht)
                    )
                    nc.scalar.activation(
                        ucc[ht][:, 0:C], ps[:, 2 * C : 3 * C], AF.Tanh, bias=bias(2, ht)
                    )
                    for half in range(2):
                        psf = ppool.tile([128, 2 * C], f32, tag="p2", bufs=1, name="psf")
                        for kk in range(2):
                            k = half * 2 + kk
                            sl = psf[:, kk * C : (kk + 1) * C]
                            fcol = 768 + ht * 128
                            for it in range(2):
                                nc.tensor.matmul(
                                    sl,
                                    lhsT=w_sb[it][:, fcol : fcol + 128],
                                    rhs=a_sb[it][:, xsl],
                                    start=(it == 0),
                                    stop=False,
                                )
                            kcol = 1792 + k * 256 + ht * 128
                            for ct in range(2):
                                nc.tensor.matmul(
                                    sl,
                                    lhsT=w_sb[ct][:, kcol : kcol + 128],
                                    rhs=hs[ct][:],
                                    start=False,
                                    stop=(ct == 1),
                                )
                        nc.scalar.activation(
                            ifff[:, C + half * 2 * C : C + (half + 1) * 2 * C],
                            psf[:],
                            AF.Sigmoid,
                            bias=bias(3, ht),
                        )
                    prod5 = gpool.tile([128, 5 * C], f16, tag=f"p5{ht}")
                    nc.vector.tensor_mul(prod5[:], ifff[:], ucc[ht][:])
                    c_sb = opool.tile([128, C], f32, tag=f"c{ht}")
                    nc.vector.tensor_reduce(
                        c_sb[:],
                        prod5[:].rearrange("p (j c) -> p c j", j=5),
                        axis=X,
                        op=ADD,
                    )
                    tc_sb = gpool.tile([128, C], f16, tag=f"tc{ht}")
                    nc.scalar.activation(tc_sb[:], c_sb[:], AF.Tanh)
                    h_sb = opool.tile([128, C], f32, tag=f"h{ht}")
                    nc.vector.tensor_mul(h_sb[:], o_sb[:], tc_sb[:])
                    rows = slice(ht * 128, (ht + 1) * 128)
                    cols = slice(c * C, (c + 1) * C)
                    nc.sync.dma_start(h_out[rows, cols], h_sb[:])
                    nc.sync.dma_start(c_out[rows, cols], c_sb[:])

            # Software pipeline: child(c) is emitted before node(c-1) so the
            # PE never stalls waiting for hsum of the current chunk.
            prev = None
            for c in range(nchunk):
                cur = emit_child(c)
                if prev is not None:
                    emit_node(prev)
                prev = cur
            emit_node(prev)

    nc.compile()
    return nc


def _prep_shared(Wi, bi, Wf, bf, Wo, bo, Wu, bu, Ui, Uo, Uu, WfK):
    """Weight/bias packing shared by all cores."""
    WxiouT = np.concatenate([Wi, Wo, Wu], axis=0).T  # [256, 768]
    WfT = np.asarray(Wf).T  # [256, 256]
    UhT = np.concatenate([Ui, Uo, Uu, WfK[0], WfK[1], WfK[2], WfK[3]], axis=0).T
    wcat = np.concatenate([WxiouT, WfT, UhT], axis=1).astype(np.float16)  # [256, 2816]
    wcat = np.ascontiguousarray(wcat.reshape(2, 128, 2816))

    bmat = np.empty((128, 8), np.float32)
    for g, b in enumerate([bi, bo, bu, bf]):
        b = np.asarray(b, np.float32)
        bmat[:, 2 * g] = b[:128]
        bmat[:, 2 * g + 1] = b[128:]
    return wcat, bmat


def _prep_core(x, child_x, m, nchunk):
    """Pack per-core activations: [nchunk, 2, 128, 5C] fp16."""
    bloc = nchunk * C
    sl = slice(m * bloc, (m + 1) * bloc)
    cxt = np.asarray(child_x[sl], np.float16).transpose(2, 1, 0)  # [256, 4, bloc]
    xt = np.asarray(x[sl], np.float16).T[:, None, :]  # [256, 1, bloc]
    full = np.concatenate([cxt, xt], axis=1)  # [256, 5, bloc]
    # [it, p, j, chunk, cb] -> [chunk, it, p, j, cb]
    full = full.reshape(2, 128, 5, nchunk, C).transpose(3, 0, 1, 2, 4)
    return np.ascontiguousarray(full).reshape(nchunk, 2, 128, 5 * C)


def _run(inputs, nchunk, trace=False):
    from concourse.bass_utils import run_bass_kernel_spmd

    key = ("nc", nchunk)
    if key not in _cache:
        _cache[key] = _build(nchunk)
    nc = _cache[key]

    wcat, bmat = _prep_shared(
        inputs["Wi"], inputs["bi"], inputs["Wf"], inputs["bf"],
        inputs["Wo"], inputs["bo"], inputs["Wu"], inputs["bu"],
        inputs["Ui"], inputs["Uo"], inputs["Uu"], inputs["WfK"],
    )
    in_maps = []
    for m in range(NCORES):
        ax = _prep_core(inputs["x"], inputs["child_x"], m, nchunk)
        in_maps.append({"ax": ax, "wcat": wcat, "bmat": bmat})

    res = run_bass_kernel_spmd(
        nc, in_maps, core_ids=list(range(NCORES)), trace=trace
    )
    bloc = nchunk * C
    h = np.empty((NCORES * bloc, 256), np.float32)
    c = np.empty((NCORES * bloc, 256), np.float32)
    for m, r in enumerate(res.results):
        h[m * bloc : (m + 1) * bloc] = r["h_out"].T
        c[m * bloc : (m + 1) * bloc] = r["c_out"].T
    return (h, c), res


def kernel(**inputs):
    (h, c), _ = _run(inputs, BLOC // C)
    return h, c


# revision 3
# speedup vs baseline: 1.1058x; 1.1058x over previous
"""NaryTreeLSTM Trainium2 kernel.

Strategy: pure data-parallel over batch (B=32768 -> 4096 rows/core on 8
cores). All on-device compute happens in transposed [h, batch] layout so
matmuls contract over the SBUF partition dim; activations are pre-cast to
fp16 host-side (halves DMA, 4x faster PE than fp32). Gate preactivations
accumulate in PSUM (x-path + hsum-path + bias via ACT), nonlinearities run
on the scalar engine straight out of PSUM in 2-bank [128,1024] tiles
(child gates merged over k-pairs, same per-partition bias), and the
k-reductions (hsum, c = i*u + sum_k f_k*cc_k) are short DVE tree-adds.
"""

import sys

sys.path.insert(0, "/opt/trn_rl_repo")

import numpy as np

B, K, I, H = 32768, 4, 256, 256
NCORES = 8
BLOC = B // NCORES  # 4096 batch rows per core
C = 512  # chunk columns (one PSUM bank of fp32)

_cache = {}


def _build(nchunk):
    """Build the per-core Bass program (identical on all cores)."""
    import concourse.bass as bass  # noqa: F401
    import concourse.tile as tile
    from concourse import bacc, mybir

    f16, f32 = mybir.dt.float16, mybir.dt.float32
    AF = mybir.ActivationFunctionType

    nc = bacc.Bacc("TRN2", target_bir_lowering=False, debug=False, num_devices=NCORES)

    # DRAM I/O. ax packs, per (chunk, itile): [cx_k0|cx_k1|cx_k2|cx_k3|x]
    # blocks of C columns each, rows = 128 contraction indices.
    ax = nc.dram_tensor("ax", [nchunk, 2, 128, 5 * C], f16, kind="ExternalInput").ap()
    # wcat cols: 0:768 WxiouT (g*256+h), 768:1024 WfT, 1024:2816 UhT
    # ([Ui|Uo|Uu|WfK0..3] at 1024+blk*256+h); rows = contraction index.
    wcat = nc.dram_tensor("wcat", [2, 128, 2816], f16, kind="ExternalInput").ap()
    # bmat cols: 2g+t for g in {i,o,u,f}, t = h-tile
    bmat = nc.dram_tensor("bmat", [128, 8], f32, kind="ExternalInput").ap()
    h_out = nc.dram_tensor("h_out", [256, nchunk * C], f32, kind="ExternalOutput").ap()
    c_out = nc.dram_tensor("c_out", [256, nchunk * C], f32, kind="ExternalOutput").ap()

    with tile.TileContext(nc) as tc:
        import contextlib

        with contextlib.ExitStack() as ctx:
            wpool = ctx.enter_context(tc.tile_pool(name="w", bufs=1))
            apool = ctx.enter_context(tc.tile_pool(name="a", bufs=3))
            gpool = ctx.enter_context(tc.tile_pool(name="g", bufs=1))
            opool = ctx.enter_context(tc.tile_pool(name="o", bufs=2))
            ppool = ctx.enter_context(tc.tile_pool(name="ps", bufs=4, space="PSUM"))

            w_sb = []
            for it in range(2):
                w = wpool.tile([128, 2816], f16, tag=f"w{it}", name=f"w{it}")
                nc.sync.dma_start(w[:], wcat[it])
                w_sb.append(w)
            b_sb = wpool.tile([128, 8], f32, tag="bias", name="b_sb")
            nc.sync.dma_start(b_sb[:], bmat[:])

            def bias(g, ht):
                # g: 0=i, 1=o, 2=u, 3=f
                return b_sb[:, 2 * g + ht : 2 * g + ht + 1]

            def psum2():
                # uniform 2-bank PSUM tiles; bufs=4 -> all 8 banks in flight
                return ppool.tile([128, 2 * C], f32, tag="p2", name="p2")

            def emit_child(c):
                """Child (leaf) phase for chunk c. Returns live tiles."""
                a_sb = []
                for it in range(2):
                    a = apool.tile([128, 5 * C], f16, tag=f"a{it}", name=f"a{it}")
                    nc.sync.dma_start(a[:], ax[c, it])
                    a_sb.append(a)
                gates = {}  # (g, ht) -> [128, 4C] fp16 (4 k-blocks)
                for ht in range(2):
                    for g in range(3):
                        gt = gpool.tile(
                            [128, 4 * C], f16, tag=f"cg{g}{ht}", bufs=2,
                            name=f"cg{g}{ht}",
                        )
                        gates[(g, ht)] = gt
                        col = g * 256 + ht * 128
                        fn = AF.Tanh if g == 2 else AF.Sigmoid
                        for half in range(2):  # k-pair (2k, 2k+1)
                            ps = psum2()
                            for kk in range(2):
                                k = half * 2 + kk
                                for it in range(2):
                                    nc.tensor.matmul(
                                        ps[:, kk * C : (kk + 1) * C],
                                        lhsT=w_sb[it][:, col : col + 128],
                                        rhs=a_sb[it][:, k * C : (k + 1) * C],
                                        start=(it == 0),
                                        stop=(it == 1),
                                    )
                            nc.scalar.activation(
                                gt[:, half * 2 * C : (half + 1) * 2 * C],
                                ps[:],
                                fn,
                                bias=bias(g, ht),
                            )
                ucc, hs = {}, {}
                for ht in range(2):
                    # ucc = [u_node | cc0..cc3]; cc written now, u in node phase
                    ucc[ht] = gpool.tile(
                        [128, 5 * C], f16, tag=f"ucc{ht}", bufs=2, name=f"ucc{ht}"
                    )
                    nc.vector.tensor_mul(
                        ucc[ht][:, C : 5 * C], gates[(0, ht)][:], gates[(2, ht)][:]
                    )
                    tcc = gpool.tile([128, 4 * C], f16, tag=f"tcc{ht}", name="tcc")
                    nc.scalar.activation(tcc[:], ucc[ht][:, C : 5 * C], AF.Tanh)
                    prod = gpool.tile([128, 4 * C], f16, tag=f"prod{ht}", name="prod")
                    nc.vector.tensor_mul(prod[:], gates[(1, ht)][:], tcc[:])
                    # hsum = sum_k prod_k, via tree adds
                    t2 = gpool.tile([128, 2 * C], f16, tag=f"t2{ht}", name="t2")
                    nc.vector.tensor_add(t2[:], prod[:, 0 : 2 * C], prod[:, 2 * C : 4 * C])
                    hs[ht] = gpool.tile(
                        [128, C], f16, tag=f"hs{ht}", bufs=2, name=f"hs{ht}"
                    )
                    with nc.allow_low_precision("hsum kept in fp16 for matmul rhs"):
                        nc.vector.tensor_add(hs[ht][:], t2[:, 0:C], t2[:, C : 2 * C])
                return dict(c=c, a=a_sb, ucc=ucc, hs=hs)

            def emit_node(st):
                c, a_sb, ucc, hs = st["c"], st["a"], st["ucc"], st["hs"]
                xsl = slice(4 * C, 5 * C)

                def xh_matmuls(sl, col, ucol, stop=True):
                    """x-path + hsum-path accumulation into PSUM slice sl."""
                    for it in range(2):
                        nc.tensor.matmul(
                            sl,
                            lhsT=w_sb[it][:, col : col + 128],
                            rhs=a_sb[it][:, xsl],
                            start=(it == 0),
                            stop=False,
                        )
                    for ct in range(2):
                        nc.tensor.matmul(
                            sl,
                            lhsT=w_sb[ct][:, ucol : ucol + 128],
                            rhs=hs[ct][:],
                            start=False,
                            stop=(ct == 1) and stop,
                        )

                for ht in range(2):
                    ifff = gpool.tile([128, 5 * C], f16, tag=f"ifff{ht}", name="ifff")
                    o_sb = gpool.tile([128, C], f16, tag=f"o{ht}", name="o_sb")
                    # i and o gates share one 2-bank tile
                    ps_io = psum2()
                    for g in range(2):
                        xh_matmuls(
                            ps_io[:, g * C : (g + 1) * C],
                            g * 256 + ht * 128,
                            1024 + g * 256 + ht * 128,
                        )
                    nc.scalar.activation(
                        ifff[:, 0:C], ps_io[:, 0:C], AF.Sigmoid, bias=bias(0, ht)
                    )
                    nc.scalar.activation(
                        o_sb[:], ps_io[:, C : 2 * C], AF.Sigmoid, bias=bias(1, ht)
                    )
                    ps_u = psum2()
                    xh_matmuls(ps_u[:, 0:C], 2 * 256 + ht * 128, 1024 + 2 * 256 + ht * 128)
                    nc.scalar.activation(
                        ucc[ht][:, 0:C], ps_u[:, 0:C], AF.Tanh, bias=bias(2, ht)
                    )
                    for half in range(2):
                        psf = psum2()
                        for kk in range(2):
                            k = half * 2 + kk
                            xh_matmuls(
                                psf[:, kk * C : (kk + 1) * C],
                                768 + ht * 128,
                                1792 + k * 256 + ht * 128,
                            )
                        nc.scalar.activation(
                            ifff[:, C + half * 2 * C : C + (half + 1) * 2 * C],
                            psf[:],
                            AF.Sigmoid,
                            bias=bias(3, ht),
                        )
                    # c = i*u + sum_k f_k*cc_k via one mult + tree adds (f32)
                    prod5 = gpool.tile([128, 5 * C], f16, tag=f"p5{ht}", name="prod5")
                    nc.vector.tensor_mul(prod5[:], ifff[:], ucc[ht][:])
                    t1 = gpool.tile([128, 2 * C], f32, tag=f"t1{ht}", name="t1")
                    nc.vector.tensor_add(
                        t1[:], prod5[:, C : 3 * C], prod5[:, 3 * C : 5 * C]
                    )
                    t2 = gpool.tile([128, C], f32, tag=f"t2n{ht}", name="t2n")
                    nc.vector.tensor_add(t2[:], t1[:, 0:C], t1[:, C : 2 * C])
                    c_sb = opool.tile([128, C], f32, tag=f"c{ht}", name="c_sb")
                    nc.vector.tensor_add(c_sb[:], t2[:], prod5[:, 0:C])
                    tc_sb = gpool.tile([128, C], f16, tag=f"tc{ht}", name="tc_sb")
                    nc.scalar.activation(tc_sb[:], c_sb[:], AF.Tanh)
                    h_sb = opool.tile([128, C], f32, tag=f"h{ht}", name="h_sb")
                    nc.vector.tensor_mul(h_sb[:], o_sb[:], tc_sb[:])
                    rows = slice(ht * 128, (ht + 1) * 128)
                    cols = slice(c * C, (c + 1) * C)
                    nc.sync.dma_start(h_out[rows, cols], h_sb[:])
                    nc.sync.dma_start(c_out[rows, cols], c_sb[:])

            # Software pipeline: child(c) is emitted before node(c-1) so the
            # PE never stalls waiting for hsum of the current chunk.
            prev = None
            for c in range(nchunk):
                cur = emit_child(c)
                if prev is not None:
                    emit_node(prev)
                prev = cur
            emit_node(prev)

    nc.compile()
    return nc


def _prep_shared(Wi, bi, Wf, bf, Wo, bo, Wu, bu, Ui, Uo, Uu, WfK):
    """Weight/bias packing shared by all cores."""
    WxiouT = np.concatenate([Wi, Wo, Wu], axis=0).T  # [256, 768]
    WfT = np.asarray(Wf).T  # [256, 256]
    UhT = np.concatenate([Ui, Uo, Uu, WfK[0], WfK[1], WfK[2], WfK[3]], axis=0).T
    wcat = np.concatenate([WxiouT, WfT, UhT], axis=1).astype(np.float16)  # [256, 2816]
    wcat = np.ascontiguousarray(wcat.reshape(2, 128, 2816))

    bmat = np.empty((128, 8), np.float32)
    for g, b in enumerate([bi, bo, bu, bf]):
        b = np.asarray(b, np.float32)
        bmat[:, 2 * g] = b[:128]
        bmat[:, 2 * g + 1] = b[128:]
    return wcat, bmat


def _prep_core(x, child_x, m, nchunk):
    """Pack per-core activations: [nchunk, 2, 128, 5C] fp16."""
    bloc = nchunk * C
    sl = slice(m * bloc, (m + 1) * bloc)
    cxt = np.asarray(child_x[sl], np.float16).transpose(2, 1, 0)  # [256, 4, bloc]
    xt = np.asarray(x[sl], np.float16).T[:, None, :]  # [256, 1, bloc]
    full = np.concatenate([cxt, xt], axis=1)  # [256, 5, bloc]
    # [it, p, j, chunk, cb] -> [chunk, it, p, j, cb]
    full = full.reshape(2, 128, 5, nchunk, C).transpose(3, 0, 1, 2, 4)
    return np.ascontiguousarray(full).reshape(nchunk, 2, 128, 5 * C)


def _run(inputs, nchunk, trace=False):
    from concourse.bass_utils import run_bass_kernel_spmd

    key = ("nc", nchunk)
    if key not in _cache:
        _cache[key] = _build(nchunk)
    nc = _cache[key]

    wcat, bmat = _prep_shared(
        inputs["Wi"], inputs["bi"], inputs["Wf"], inputs["bf"],
        inputs["Wo"], inputs["bo"], inputs["Wu"], inputs["bu"],
        inputs["Ui"], inputs["Uo"], inputs["Uu"], inputs["WfK"],
    )
    in_maps = []
    for m in range(NCORES):
        ax = _prep_core(inputs["x"], inputs["child_x"], m, nchunk)
        in_maps.append({"ax": ax, "wcat": wcat, "bmat": bmat})

    res = run_bass_kernel_spmd(
        nc, in_maps, core_ids=list(range(NCORES)), trace=trace
    )
    bloc = nchunk * C
    h = np.empty((NCORES * bloc, 256), np.float32)
    c = np.empty((NCORES * bloc, 256), np.float32)
    for m, r in enumerate(res.results):
        h[m * bloc : (m + 1) * bloc] = r["h_out"].T
        c[m * bloc : (m + 1) * bloc] = r["c_out"].T
    return (h, c), res


def kernel(**inputs):
    (h, c), _ = _run(inputs, BLOC // C)
    return h, c


# revision 5
# speedup vs baseline: 1.2637x; 1.1428x over previous
"""NaryTreeLSTM Trainium2 kernel.

Strategy: pure data-parallel over batch (B=32768 -> 4096 rows/core on 8
cores). All on-device compute happens in transposed [h, batch] layout so
matmuls contract over the SBUF partition dim; activations are pre-cast to
fp16 host-side (halves DMA, 4x faster PE than fp32). Gate preactivations
accumulate in PSUM (x-path + hsum-path + bias via ACT), nonlinearities run
on the scalar engine straight out of PSUM in 2-bank [128,1024] tiles
(child gates merged over k-pairs, same per-partition bias), and the
k-reductions (hsum, c = i*u + sum_k f_k*cc_k) are short DVE tree-adds.
"""

import sys

sys.path.insert(0, "/opt/trn_rl_repo")

import numpy as np

B, K, I, H = 32768, 4, 256, 256
NCORES = 8
BLOC = B // NCORES  # 4096 batch rows per core
C = 512  # chunk columns (one PSUM bank of fp32)

_cache = {}


def _build(nchunk):
    """Build the per-core Bass program (identical on all cores)."""
    import concourse.bass as bass  # noqa: F401
    import concourse.tile as tile
    from concourse import bacc, mybir

    f16, f32 = mybir.dt.float16, mybir.dt.float32
    AF = mybir.ActivationFunctionType

    nc = bacc.Bacc("TRN2", target_bir_lowering=False, debug=False, num_devices=NCORES)

    # DRAM I/O. ax packs, per (chunk, itile): [cx_k0|cx_k1|cx_k2|cx_k3|x]
    # blocks of C columns each, rows = 128 contraction indices.
    ax = nc.dram_tensor("ax", [nchunk, 2, 128, 5 * C], f16, kind="ExternalInput").ap()
    # wcat cols: 0:768 WxiouT (g*256+h), 768:1024 WfT, 1024:2816 UhT
    # ([Ui|Uo|Uu|WfK0..3] at 1024+blk*256+h); rows = contraction index.
    wcat = nc.dram_tensor("wcat", [2, 128, 2816], f16, kind="ExternalInput").ap()
    # bmat cols: 2g+t for g in {i,o,u,f}, t = h-tile
    bmat = nc.dram_tensor("bmat", [128, 8], f32, kind="ExternalInput").ap()
    h_out = nc.dram_tensor("h_out", [256, nchunk * C], f32, kind="ExternalOutput").ap()
    c_out = nc.dram_tensor("c_out", [256, nchunk * C], f32, kind="ExternalOutput").ap()

    with tile.TileContext(nc) as tc:
        import contextlib

        with contextlib.ExitStack() as ctx:
            wpool = ctx.enter_context(tc.tile_pool(name="w", bufs=1))
            apool = ctx.enter_context(tc.tile_pool(name="a", bufs=3))
            gpool = ctx.enter_context(tc.tile_pool(name="g", bufs=1))
            opool = ctx.enter_context(tc.tile_pool(name="o", bufs=2))
            ppool = ctx.enter_context(tc.tile_pool(name="ps", bufs=2, space="PSUM"))

            # weights on the gpsimd DMA queue so the first ax load (sync
            # queue) runs concurrently; x-path weights first so child
            # matmuls can start before the U-path weights arrive.
            wA, wB = [], []
            for it in range(2):
                a_ = wpool.tile([128, 1024], f16, tag=f"wA{it}", name=f"wA{it}")
                nc.gpsimd.dma_start(a_[:], wcat[it, :, 0:1024])
                wA.append(a_)
            for it in range(2):
                b_ = wpool.tile([128, 1792], f16, tag=f"wB{it}", name=f"wB{it}")
                nc.gpsimd.dma_start(b_[:], wcat[it, :, 1024:2816])
                wB.append(b_)
            b_sb = wpool.tile([128, 8], f32, tag="bias", name="b_sb")
            nc.gpsimd.dma_start(b_sb[:], bmat[:])

            def wx(it, col):
                return wA[it][:, col : col + 128]

            def wu(it, col):
                return wB[it][:, col - 1024 : col - 1024 + 128]

            def bias(g, ht):
                # g: 0=i, 1=o, 2=u, 3=f
                return b_sb[:, 2 * g + ht : 2 * g + ht + 1]

            def psum4():
                # uniform 4-bank PSUM tiles; bufs=2 -> all 8 banks in flight
                return ppool.tile([128, 4 * C], f32, tag="p4", name="p4")

            def emit_child(c):
                """Child (leaf) phase for chunk c. Returns live tiles."""
                a_sb = []
                for it in range(2):
                    a = apool.tile([128, 5 * C], f16, tag=f"a{it}", name=f"a{it}")
                    nc.sync.dma_start(a[:], ax[c, it])
                    a_sb.append(a)
                gates = {}  # (g, ht) -> [128, 4C] fp16 (4 k-blocks)
                for ht in range(2):
                    for g in range(3):
                        gt = gpool.tile(
                            [128, 4 * C], f16, tag=f"cg{g}{ht}", bufs=2,
                            name=f"cg{g}{ht}",
                        )
                        gates[(g, ht)] = gt
                        col = g * 256 + ht * 128
                        fn = AF.Tanh if g == 2 else AF.Sigmoid
                        ps = psum4()
                        for it in range(2):  # it-major: same lhsT for 4 MMs
                            for k in range(4):
                                nc.tensor.matmul(
                                    ps[:, k * C : (k + 1) * C],
                                    lhsT=wx(it, col),
                                    rhs=a_sb[it][:, k * C : (k + 1) * C],
                                    start=(it == 0),
                                    stop=(it == 1),
                                )
                        nc.scalar.activation(gt[:], ps[:], fn, bias=bias(g, ht))
                ucc, hs = {}, {}
                for ht in range(2):
                    # ucc = [u_node | cc0..cc3]; cc written now, u in node phase
                    ucc[ht] = gpool.tile(
                        [128, 5 * C], f16, tag=f"ucc{ht}", bufs=2, name=f"ucc{ht}"
                    )
                    nc.vector.tensor_mul(
                        ucc[ht][:, C : 5 * C], gates[(0, ht)][:], gates[(2, ht)][:]
                    )
                    tcc = gpool.tile([128, 4 * C], f16, tag=f"tcc{ht}", name="tcc")
                    nc.scalar.activation(tcc[:], ucc[ht][:, C : 5 * C], AF.Tanh)
                    prod = gpool.tile([128, 4 * C], f16, tag=f"prod{ht}", name="prod")
                    nc.vector.tensor_mul(prod[:], gates[(1, ht)][:], tcc[:])
                    # hsum = sum_k prod_k, via tree adds
                    t2 = gpool.tile([128, 2 * C], f16, tag=f"t2{ht}", name="t2")
                    nc.vector.tensor_add(t2[:], prod[:, 0 : 2 * C], prod[:, 2 * C : 4 * C])
                    hs[ht] = gpool.tile(
                        [128, C], f16, tag=f"hs{ht}", bufs=2, name=f"hs{ht}"
                    )
                    with nc.allow_low_precision("hsum kept in fp16 for matmul rhs"):
                        nc.vector.tensor_add(hs[ht][:], t2[:, 0:C], t2[:, C : 2 * C])
                return dict(c=c, a=a_sb, ucc=ucc, hs=hs)

            def emit_node(st):
                c, a_sb, ucc, hs = st["c"], st["a"], st["ucc"], st["hs"]
                xsl = slice(4 * C, 5 * C)

                for ht in range(2):
                    ifff = gpool.tile([128, 5 * C], f16, tag=f"ifff{ht}", name="ifff")
                    o_sb = gpool.tile([128, C], f16, tag=f"o{ht}", name="o_sb")
                    # i, o, u gates share one 4-bank tile [i|o|u|unused]
                    ps_iou = psum4()
                    for it in range(2):
                        for g in range(3):
                            nc.tensor.matmul(
                                ps_iou[:, g * C : (g + 1) * C],
                                lhsT=wx(it, g * 256 + ht * 128),
                                rhs=a_sb[it][:, xsl],
                                start=(it == 0),
                                stop=False,
                            )
                    for ct in range(2):
                        for g in range(3):
                            nc.tensor.matmul(
                                ps_iou[:, g * C : (g + 1) * C],
                                lhsT=wu(ct, 1024 + g * 256 + ht * 128),
                                rhs=hs[ct][:],
                                start=False,
                                stop=(ct == 1),
                            )
                    nc.scalar.activation(
                        ifff[:, 0:C], ps_iou[:, 0:C], AF.Sigmoid, bias=bias(0, ht)
                    )
                    nc.scalar.activation(
                        o_sb[:], ps_iou[:, C : 2 * C], AF.Sigmoid, bias=bias(1, ht)
                    )
                    nc.scalar.activation(
                        ucc[ht][:, 0:C], ps_iou[:, 2 * C : 3 * C], AF.Tanh,
                        bias=bias(2, ht),
                    )
                    # f gates for all 4 children in one 4-bank tile
                    psf = psum4()
                    for it in range(2):
                        for k in range(4):
                            nc.tensor.matmul(
                                psf[:, k * C : (k + 1) * C],
                                lhsT=wx(it, 768 + ht * 128),
                                rhs=a_sb[it][:, xsl],
                                start=(it == 0),
                                stop=False,
                            )
                    for ct in range(2):
                        for k in range(4):
                            nc.tensor.matmul(
                                psf[:, k * C : (k + 1) * C],
                                lhsT=wu(ct, 1792 + k * 256 + ht * 128),
                                rhs=hs[ct][:],
                                start=False,
                                stop=(ct == 1),
                            )
                    nc.scalar.activation(
                        ifff[:, C : 5 * C], psf[:], AF.Sigmoid, bias=bias(3, ht)
                    )
                    # c = i*u + sum_k f_k*cc_k via one mult + tree adds (f32)
                    prod5 = gpool.tile([128, 5 * C], f16, tag=f"p5{ht}", name="prod5")
                    nc.vector.tensor_mul(prod5[:], ifff[:], ucc[ht][:])
                    t1 = gpool.tile([128, 2 * C], f32, tag=f"t1{ht}", name="t1")
                    nc.vector.tensor_add(
                        t1[:], prod5[:, C : 3 * C], prod5[:, 3 * C : 5 * C]
                    )
                    t2 = gpool.tile([128, C], f32, tag=f"t2n{ht}", name="t2n")
                    nc.vector.tensor_add(t2[:], t1[:, 0:C], t1[:, C : 2 * C])
                    c_sb = opool.tile([128, C], f32, tag=f"c{ht}", name="c_sb")
                    nc.vector.tensor_add(c_sb[:], t2[:], prod5[:, 0:C])
                    tc_sb = gpool.tile([128, C], f16, tag=f"tc{ht}", name="tc_sb")
                    nc.scalar.activation(tc_sb[:], c_sb[:], AF.Tanh)
                    h_sb = opool.tile([128, C], f32, tag=f"h{ht}", name="h_sb")
                    nc.vector.tensor_mul(h_sb[:], o_sb[:], tc_sb[:])
                    rows = slice(ht * 128, (ht + 1) * 128)
                    cols = slice(c * C, (c + 1) * C)
                    nc.sync.dma_start(h_out[rows, cols], h_sb[:])
                    nc.sync.dma_start(c_out[rows, cols], c_sb[:])

            # Software pipeline: child(c) is emitted before node(c-1) so the
            # PE never stalls waiting for hsum of the current chunk.
            prev = None
            for c in range(nchunk):
                cur = emit_child(c)
                if prev is not None:
                    emit_node(prev)
                prev = cur
            emit_node(prev)

    nc.compile()
    return nc


def _prep_shared(Wi, bi, Wf, bf, Wo, bo, Wu, bu, Ui, Uo, Uu, WfK):
    """Weight/bias packing shared by all cores."""
    WxiouT = np.concatenate([Wi, Wo, Wu], axis=0).T  # [256, 768]
    WfT = np.asarray(Wf).T  # [256, 256]
    UhT = np.concatenate([Ui, Uo, Uu, WfK[0], WfK[1], WfK[2], WfK[3]], axis=0).T
    wcat = np.concatenate([WxiouT, WfT, UhT], axis=1).astype(np.float16)  # [256, 2816]
    wcat = np.ascontiguousarray(wcat.reshape(2, 128, 2816))

    bmat = np.empty((128, 8), np.float32)
    for g, b in enumerate([bi, bo, bu, bf]):
        b = np.asarray(b, np.float32)
        bmat[:, 2 * g] = b[:128]
        bmat[:, 2 * g + 1] = b[128:]
    return wcat, bmat


def _prep_core(x, child_x, m, nchunk):
    """Pack per-core activations: [nchunk, 2, 128, 5C] fp16."""
    bloc = nchunk * C
    sl = slice(m * bloc, (m + 1) * bloc)
    cxt = np.asarray(child_x[sl], np.float16).transpose(2, 1, 0)  # [256, 4, bloc]
    xt = np.asarray(x[sl], np.float16).T[:, None, :]  # [256, 1, bloc]
    full = np.concatenate([cxt, xt], axis=1)  # [256, 5, bloc]
    # [it, p, j, chunk, cb] -> [chunk, it, p, j, cb]
    full = full.reshape(2, 128, 5, nchunk, C).transpose(3, 0, 1, 2, 4)
    return np.ascontiguousarray(full).reshape(nchunk, 2, 128, 5 * C)


def _run(inputs, nchunk, trace=False):
    from concourse.bass_utils import run_bass_kernel_spmd

    key = ("nc", nchunk)
    if key not in _cache:
        _cache[key] = _build(nchunk)
    nc = _cache[key]

    wcat, bmat = _prep_shared(
        inputs["Wi"], inputs["bi"], inputs["Wf"], inputs["bf"],
        inputs["Wo"], inputs["bo"], inputs["Wu"], inputs["bu"],
        inputs["Ui"], inputs["Uo"], inputs["Uu"], inputs["WfK"],
    )
    in_maps = []
    for m in range(NCORES):
        ax = _prep_core(inputs["x"], inputs["child_x"], m, nchunk)
        in_maps.append({"ax": ax, "wcat": wcat, "bmat": bmat})

    res = run_bass_kernel_spmd(
        nc, in_maps, core_ids=list(range(NCORES)), trace=trace
    )
    bloc = nchunk * C
    h = np.empty((NCORES * bloc, 256), np.float32)
    c = np.empty((NCORES * bloc, 256), np.float32)
    for m, r in enumerate(res.results):
        h[m * bloc : (m + 1) * bloc] = r["h_out"].T
        c[m * bloc : (m + 1) * bloc] = r["c_out"].T
    return (h, c), res


def kernel(**inputs):
    (h, c), _ = _run(inputs, BLOC // C)
    return h, c


# revision 8
# speedup vs baseline: 1.2730x; 1.0074x over previous
"""NaryTreeLSTM Trainium2 kernel.

Strategy: pure data-parallel over batch (B=32768 -> 4096 rows/core on 8
cores). All on-device compute happens in transposed [h, batch] layout so
matmuls contract over the SBUF partition dim; activations are pre-cast to
fp16 host-side (halves DMA, 4x faster PE than fp32). Gate preactivations
accumulate in PSUM (x-path + hsum-path + bias via ACT), nonlinearities run
on the scalar engine straight out of PSUM in 2-bank [128,1024] tiles
(child gates merged over k-pairs, same per-partition bias), and the
k-reductions (hsum, c = i*u + sum_k f_k*cc_k) are short DVE tree-adds.
"""

import sys

sys.path.insert(0, "/opt/trn_rl_repo")

import numpy as np

B, K, I, H = 32768, 4, 256, 256
NCORES = 8
BLOC = B // NCORES  # 4096 batch rows per core
C = 512  # chunk columns (one PSUM bank of fp32)

_cache = {}


def _build(nchunk):
    """Build the per-core Bass program (identical on all cores)."""
    import concourse.bass as bass  # noqa: F401
    import bass_rust as _bass_rust
    import concourse.tile as tile
    from concourse import bacc, mybir

    f16, f32 = mybir.dt.float16, mybir.dt.float32
    AF = mybir.ActivationFunctionType

    nc = bacc.Bacc("TRN2", target_bir_lowering=False, debug=False, num_devices=NCORES)

    # DRAM I/O. ax packs, per (chunk, itile): [cx_k0|cx_k1|cx_k2|cx_k3|x]
    # blocks of C columns each, rows = 128 contraction indices.
    ax = nc.dram_tensor("ax", [nchunk, 2, 128, 5 * C], f16, kind="ExternalInput").ap()
    # wcat cols: 0:768 WxiouT (g*256+h), 768:1024 WfT, 1024:2816 UhT
    # ([Ui|Uo|Uu|WfK0..3] at 1024+blk*256+h); rows = contraction index.
    wcat = nc.dram_tensor("wcat", [2, 128, 2816], f16, kind="ExternalInput").ap()
    # bmat cols: 2g+t for g in {i,o,u,f}, t = h-tile
    bmat = nc.dram_tensor("bmat", [128, 8], f32, kind="ExternalInput").ap()
    h_out = nc.dram_tensor("h_out", [256, nchunk * C], f32, kind="ExternalOutput").ap()
    c_out = nc.dram_tensor("c_out", [256, nchunk * C], f32, kind="ExternalOutput").ap()

    with tile.TileContext(nc) as tc:
        import contextlib

        with contextlib.ExitStack() as ctx:
            wpool = ctx.enter_context(tc.tile_pool(name="w", bufs=1))
            apool = ctx.enter_context(tc.tile_pool(name="a", bufs=3))
            gpool = ctx.enter_context(tc.tile_pool(name="g", bufs=1))
            opool = ctx.enter_context(tc.tile_pool(name="o", bufs=2))
            ppool = ctx.enter_context(tc.tile_pool(name="ps", bufs=2, space="PSUM"))

            # weights on the gpsimd DMA queue so the first ax load (sync
            # queue) runs concurrently; x-path weights first so child
            # matmuls can start before the U-path weights arrive.
            wA, wB = [], []
            for it in range(2):
                a_ = wpool.tile([128, 1024], f16, tag=f"wA{it}", name=f"wA{it}")
                nc.gpsimd.dma_start(a_[:], wcat[it, :, 0:1024])
                wA.append(a_)
            for it in range(2):
                b_ = wpool.tile([128, 1792], f16, tag=f"wB{it}", name=f"wB{it}")
                nc.gpsimd.dma_start(b_[:], wcat[it, :, 1024:2816])
                wB.append(b_)
            b_sb = wpool.tile([128, 8], f32, tag="bias", name="b_sb")
            nc.gpsimd.dma_start(b_sb[:], bmat[:])
            # tiny dummy activations so the ACT table load (~1.3us) happens
            # during the initial DMA wait instead of before the first gate
            warm = wpool.tile([1, 8], f32, tag="warm", name="warm")
            nc.vector.memset(warm[:], 0.0)
            nc.scalar.activation(warm[:], warm[:], AF.Sigmoid)

            def wx(it, col):
                return wA[it][:, col : col + 128]

            def wu(it, col):
                return wB[it][:, col - 1024 : col - 1024 + 128]

            def bias(g, ht):
                # g: 0=i, 1=o, 2=u, 3=f
                return b_sb[:, 2 * g + ht : 2 * g + ht + 1]

            def psum4():
                # uniform 4-bank PSUM tiles; bufs=2 -> all 8 banks in flight
                return ppool.tile([128, 4 * C], f32, tag="p4", name="p4")

            def emit_child(c):
                """Child (leaf) phase for chunk c. Returns live tiles."""
                a_sb = []
                for it in range(2):
                    a = apool.tile([128, 5 * C], f16, tag=f"a{it}", name=f"a{it}")
                    nc.sync.dma_start(a[:], ax[c, it])
                    a_sb.append(a)
                gates = {}  # (g, ht) -> [128, 4C] fp16 (4 k-blocks)
                for ht in range(2):
                    for g in range(3):
                        gt = gpool.tile(
                            [128, 4 * C], f16, tag=f"cg{g}{ht}", bufs=2,
                            name=f"cg{g}{ht}",
                        )
                        gates[(g, ht)] = gt
                        col = g * 256 + ht * 128
                        fn = AF.Tanh if g == 2 else AF.Sigmoid
                        ps = psum4()
                        for it in range(2):  # it-major: same lhsT for 4 MMs
                            for k in range(4):
                                nc.tensor.matmul(
                                    ps[:, k * C : (k + 1) * C],
                                    lhsT=wx(it, col),
                                    rhs=a_sb[it][:, k * C : (k + 1) * C],
                                    start=(it == 0),
                                    stop=(it == 1),
                                )
                        nc.scalar.activation(gt[:], ps[:], fn, bias=bias(g, ht))
                return dict(c=c, a=a_sb, gates=gates)

            def emit_child_tail(st):
                gates = st["gates"]
                ucc, hs = {}, {}
                for ht in range(2):
                    # ucc = [u_node | cc0..cc3]; cc written now, u in node phase
                    ucc[ht] = gpool.tile(
                        [128, 5 * C], f16, tag=f"ucc{ht}", bufs=2, name=f"ucc{ht}"
                    )
                    nc.vector.tensor_mul(
                        ucc[ht][:, C : 5 * C], gates[(0, ht)][:], gates[(2, ht)][:]
                    )
                    tcc = gpool.tile([128, 4 * C], f16, tag=f"tcc{ht}", name="tcc")
                    nc.scalar.activation(tcc[:], ucc[ht][:, C : 5 * C], AF.Tanh)
                    prod = gpool.tile([128, 4 * C], f16, tag=f"prod{ht}", name="prod")
                    nc.vector.tensor_mul(prod[:], gates[(1, ht)][:], tcc[:])
                    # hsum = sum_k prod_k, via tree adds
                    t2 = gpool.tile([128, 2 * C], f16, tag=f"t2{ht}", name="t2")
                    nc.vector.tensor_add(t2[:], prod[:, 0 : 2 * C], prod[:, 2 * C : 4 * C])
                    hs[ht] = gpool.tile(
                        [128, C], f16, tag=f"hs{ht}", bufs=2, name=f"hs{ht}"
                    )
                    with nc.allow_low_precision("hsum kept in fp16 for matmul rhs"):
                        nc.vector.tensor_add(hs[ht][:], t2[:, 0:C], t2[:, C : 2 * C])
                st["ucc"] = ucc
                st["hs"] = hs

            def emit_node(st):
                c, a_sb, ucc, hs = st["c"], st["a"], st["ucc"], st["hs"]
                xsl = slice(4 * C, 5 * C)

                for ht in range(2):
                    ifff = gpool.tile([128, 5 * C], f16, tag=f"ifff{ht}", name="ifff")
                    o_sb = gpool.tile([128, C], f16, tag=f"o{ht}", name="o_sb")
                    # i, o, u gates share one 4-bank tile [i|o|u|unused]
                    ps_iou = psum4()
                    for it in range(2):
                        for g in range(3):
                            nc.tensor.matmul(
                                ps_iou[:, g * C : (g + 1) * C],
                                lhsT=wx(it, g * 256 + ht * 128),
                                rhs=a_sb[it][:, xsl],
                                start=(it == 0),
                                stop=False,
                            )
                    for ct in range(2):
                        for g in range(3):
                            nc.tensor.matmul(
                                ps_iou[:, g * C : (g + 1) * C],
                                lhsT=wu(ct, 1024 + g * 256 + ht * 128),
                                rhs=hs[ct][:],
                                start=False,
                                stop=(ct == 1),
                            )
                    nc.scalar.activation(
                        ifff[:, 0:C], ps_iou[:, 0:C], AF.Sigmoid, bias=bias(0, ht)
                    )
                    nc.scalar.activation(
                        o_sb[:], ps_iou[:, C : 2 * C], AF.Sigmoid, bias=bias(1, ht)
                    )
                    nc.scalar.activation(
                        ucc[ht][:, 0:C], ps_iou[:, 2 * C : 3 * C], AF.Tanh,
                        bias=bias(2, ht),
                    )
                    # f gates for all 4 children in one 4-bank tile
                    psf = psum4()
                    for it in range(2):
                        for k in range(4):
                            nc.tensor.matmul(
                                psf[:, k * C : (k + 1) * C],
                                lhsT=wx(it, 768 + ht * 128),
                                rhs=a_sb[it][:, xsl],
                                start=(it == 0),
                                stop=False,
                            )
                    for ct in range(2):
                        for k in range(4):
                            nc.tensor.matmul(
                                psf[:, k * C : (k + 1) * C],
                                lhsT=wu(ct, 1792 + k * 256 + ht * 128),
                                rhs=hs[ct][:],
                                start=False,
                                stop=(ct == 1),
                            )
                    nc.scalar.activation(
                        ifff[:, C : 5 * C], psf[:], AF.Sigmoid, bias=bias(3, ht)
                    )
                    # c = i*u + sum_k f_k*cc_k via one mult + tree adds (f32)
                    prod5 = gpool.tile([128, 5 * C], f16, tag=f"p5{ht}", name="prod5")
                    nc.vector.tensor_mul(prod5[:], ifff[:], ucc[ht][:])
                    t1 = gpool.tile([128, 2 * C], f32, tag=f"t1{ht}", name="t1")
                    nc.vector.tensor_add(
                        t1[:], prod5[:, C : 3 * C], prod5[:, 3 * C : 5 * C]
                    )
                    t2 = gpool.tile([128, C], f32, tag=f"t2n{ht}", name="t2n")
                    nc.vector.tensor_add(t2[:], t1[:, 0:C], t1[:, C : 2 * C])
                    c_sb = opool.tile([128, C], f32, tag=f"c{ht}", name="c_sb")
                    nc.vector.tensor_add(c_sb[:], t2[:], prod5[:, 0:C])
                    tc_sb = gpool.tile([128, C], f16, tag=f"tc{ht}", name="tc_sb")
                    nc.scalar.activation(tc_sb[:], c_sb[:], AF.Tanh)
                    h_sb = opool.tile([128, C], f32, tag=f"h{ht}", name="h_sb")
                    nc.vector.tensor_mul(h_sb[:], o_sb[:], tc_sb[:])
                    rows = slice(ht * 128, (ht + 1) * 128)
                    cols = slice(c * C, (c + 1) * C)
                    nc.sync.dma_start(h_out[rows, cols], h_sb[:])
                    nc.sync.dma_start(c_out[rows, cols], c_sb[:])

            # Software pipeline: child_gates(c) | node(c-1) | child_tail(c)
            # so ACT never waits on the DVE cc-chain and the PE never waits
            # on hsum of the current chunk.
            prev = None
            for c in range(nchunk):
                cur = emit_child(c)
                if prev is not None:
                    emit_node(prev)
                emit_child_tail(cur)
                prev = cur
            emit_node(prev)

    nc.compile()
    return nc


def _prep_shared(Wi, bi, Wf, bf, Wo, bo, Wu, bu, Ui, Uo, Uu, WfK):
    """Weight/bias packing shared by all cores."""
    WxiouT = np.concatenate([Wi, Wo, Wu], axis=0).T  # [256, 768]
    WfT = np.asarray(Wf).T  # [256, 256]
    UhT = np.concatenate([Ui, Uo, Uu, WfK[0], WfK[1], WfK[2], WfK[3]], axis=0).T
    wcat = np.concatenate([WxiouT, WfT, UhT], axis=1).astype(np.float16)  # [256, 2816]
    wcat = np.ascontiguousarray(wcat.reshape(2, 128, 2816))

    bmat = np.empty((128, 8), np.float32)
    for g, b in enumerate([bi, bo, bu, bf]):
        b = np.asarray(b, np.float32)
        bmat[:, 2 * g] = b[:128]
        bmat[:, 2 * g + 1] = b[128:]
    return wcat, bmat


def _prep_core(x, child_x, m, nchunk):
    """Pack per-core activations: [nchunk, 2, 128, 5C] fp16."""
    bloc = nchunk * C
    sl = slice(m * bloc, (m + 1) * bloc)
    cxt = np.asarray(child_x[sl], np.float16).transpose(2, 1, 0)  # [256, 4, bloc]
    xt = np.asarray(x[sl], np.float16).T[:, None, :]  # [256, 1, bloc]
    full = np.concatenate([cxt, xt], axis=1)  # [256, 5, bloc]
    # [it, p, j, chunk, cb] -> [chunk, it, p, j, cb]
    full = full.reshape(2, 128, 5, nchunk, C).transpose(3, 0, 1, 2, 4)
    return np.ascontiguousarray(full).reshape(nchunk, 2, 128, 5 * C)


def _run(inputs, nchunk, trace=False):
    from concourse.bass_utils import run_bass_kernel_spmd

    key = ("nc", nchunk)
    if key not in _cache:
        _cache[key] = _build(nchunk)
    nc = _cache[key]

    wcat, bmat = _prep_shared(
        inputs["Wi"], inputs["bi"], inputs["Wf"], inputs["bf"],
        inputs["Wo"], inputs["bo"], inputs["Wu"], inputs["bu"],
        inputs["Ui"], inputs["Uo"], inputs["Uu"], inputs["WfK"],
    )
    in_maps = []
    for m in range(NCORES):
        ax = _prep_core(inputs["x"], inputs["child_x"], m, nchunk)
        in_maps.append({"ax": ax, "wcat": wcat, "bmat": bmat})

    res = run_bass_kernel_spmd(
        nc, in_maps, core_ids=list(range(NCORES)), trace=trace
    )
    bloc = nchunk * C
    h = np.empty((NCORES * bloc, 256), np.float32)
    c = np.empty((NCORES * bloc, 256), np.float32)
    for m, r in enumerate(res.results):
        h[m * bloc : (m + 1) * bloc] = r["h_out"].T
        c[m * bloc : (m + 1) * bloc] = r["c_out"].T
    return (h, c), res


def kernel(**inputs):
    (h, c), _ = _run(inputs, BLOC // C)
    return h, c
